# revision 28
# baseline (speedup 1.0000x reference)
"""Butterworth IIR (order 4) over [B=128, T=160000, 1] on 8 TRN2 NeuronCores.

Strategy: a stable IIR's impulse response decays geometrically (max pole
radius ~0.668 here), so the filter is numerically exactly (tail < 3e-23)
a 128-tap causal FIR:  y[t] = sum_{k<128} h[k] x[t-k].

Chunking time into 128-sample chunks, with X[c, m] = x[c*128 + m]:
    y[c*128 + j] = sum_m X[c, m] H0[m, j] + sum_m X[c-1, m] H1[m, j]
    H0[m, j] = h[j - m]        (0 <= j - m < 128)
    H1[m, j] = h[j - m + 128]  (0 <= j - m + 128 < 128)

On device this is two accumulating TensorE matmuls per window with the
small fixed H matrices as the stationary operand and a phase-major
(transposed) view of x as the wide moving operand (N up to 512 chunks):
    psum[j, n] = sum_m H0[m, j] XT[m, w+1+n] + sum_m H1[m, j] XT[m, w+n]
The host supplies XT (phase-major) with a leading zero column per
sequence so chunk 0 sees zeros as its predecessor, and un-transposes the
phase-major output.

This is HBM-bandwidth bound (~358 GB/s per core), so I/O is minimized:
f16 both ways (output cast f32->f16 on-chip during PSUM evacuation,
widened back to f32 on the host; rel-err budget 2e-2 vs ~4e-4 achieved)
and moved with few, large DMAs. PSUM evacuation alternates between the
Scalar and Vector engines so neither becomes the bottleneck; output DMAs
alternate between the scalar HWDGE ring and the gpsimd SWDGE ring while
the sync HWDGE ring streams the input.

Sharding: pure data-parallel, batch 128 -> 16 sequences per core.
"""

import numpy as np

B_FULL = 128
T_FULL = 160000
N_CORES = 8
SEQ_PER_CORE = B_FULL // N_CORES  # 16
CHUNK = 128
NCHUNK = T_FULL // CHUNK  # 1250
TAPS = 128
NWIN = 512  # matmul moving-operand width (chunks per matmul) = 1 PSUM bank
XSTRIDE = NCHUNK + 1  # per-seq column block in xt (leading zero column)
XCOLS = SEQ_PER_CORE * XSTRIDE  # 20016
YCOLS = SEQ_PER_CORE * NCHUNK  # 20000

_NC_CACHE = {}


def _impulse_response(b, a, n):
    """First n samples of the IIR impulse response, computed in float64
    via the same direct-form II transposed recurrence as the reference."""
    b = np.asarray(b, np.float64)
    a = np.asarray(a, np.float64)
    bn = b / a[0]
    an = a / a[0]
    order = len(a) - 1
    z = np.zeros(order, np.float64)
    h = np.zeros(n, np.float64)
    xt = 1.0
    for t in range(n):
        yt = bn[0] * xt + z[0]
        znew = np.empty_like(z)
        znew[:-1] = z[1:] + xt * bn[1:-1] - yt * an[1:-1]
        znew[-1] = xt * bn[-1] - yt * an[-1]
        z = znew
        h[t] = yt
        xt = 0.0
    return h


def _build_h_matrices(b, a):
    h = _impulse_response(b, a, TAPS)
    m = np.arange(CHUNK)[:, None]
    j = np.arange(CHUNK)[None, :]
    d0 = j - m
    d1 = j - m + CHUNK
    H0 = np.where((d0 >= 0) & (d0 < TAPS), h[np.clip(d0, 0, TAPS - 1)], 0.0)
    H1 = np.where((d1 >= 0) & (d1 < TAPS), h[np.clip(d1, 0, TAPS - 1)], 0.0)
    return np.concatenate([H0, H1], axis=1).astype(np.float16)  # [128, 256]


def _build_nc():
    import concourse.bacc as bacc
    import concourse.mybir as mybir
    from concourse.tile import TileContext

    f32 = mybir.dt.float32
    f16 = mybir.dt.float16
    nc = bacc.Bacc()
    xt = nc.declare_dram_parameter("xt", [CHUNK, XCOLS], f16, isOutput=False)
    hh = nc.declare_dram_parameter("hh", [CHUNK, 2 * CHUNK], f16, isOutput=False)
    yt = nc.declare_dram_parameter("yt", [CHUNK, YCOLS], f16, isOutput=True)

    with TileContext(nc) as tc:
        with (
            tc.tile_pool(name="const", bufs=1) as cpool,
            tc.tile_pool(name="xin", bufs=1) as xpool,
            tc.tile_pool(name="yout", bufs=1) as ypool,
            tc.tile_pool(name="warm", bufs=1) as wpool,
            tc.tile_pool(name="acc", bufs=2, space="PSUM") as pspool,
        ):
            h_tile = cpool.tile([CHUNK, 2 * CHUNK], f16)
            nc.sync.dma_start(out=h_tile[:], in_=hh[:])

            # Whole input and output stay resident in SBUF (~10.3 MB of 26).
            x_tile = xpool.tile([CHUNK, XCOLS], f16)
            y_tile = ypool.tile([CHUNK, YCOLS], f16)

            # Stream the input over the sync HWDGE ring (seqs 0-7) and the
            # gpsimd SWDGE ring (seqs 8-15). One HWDGE ring alone is limited
            # to ~250-300 GB/s (input takes ~20us and stalls late sequences),
            # and two HWDGE rings falsely couple through the shared DMAHW
            # completion lanes - but SWDGE completions tick a separate proc,
            # so this split halves input time without false ordering.
            nc.sync.dma_start(out=x_tile[:, : NWIN + 1], in_=xt[:, : NWIN + 1])
            nc.sync.dma_start(
                out=x_tile[:, NWIN + 1 : XSTRIDE], in_=xt[:, NWIN + 1 : XSTRIDE]
            )
            s0 = 1
            for nseq, eng in (
                (1, nc.sync),
                (2, nc.sync),
                (4, nc.sync),
                (4, nc.gpsimd),
                (4, nc.gpsimd),
            ):
                c0, c1 = s0 * XSTRIDE, (s0 + nseq) * XSTRIDE
                eng.dma_start(out=x_tile[:, c0:c1], in_=xt[:, c0:c1])
                s0 += nseq

            # Warm the PE clock: the HAM up-clocks 1.2->2.4 GHz only after
            # ~3.4us of sustained activity, so burn dummy matmuls (no DMA
            # deps: operands come from a memset tile) while the first input
            # transfer is still in flight. Without this, the first two
            # sequences run at half clock and delay the whole pipeline.
            warm = wpool.tile([CHUNK, NWIN], f16)
            nc.vector.memset(warm[:], 0.0)
            wps = pspool.tile([CHUNK, 2 * NWIN], f32, tag="big", bufs=3)
            for _ in range(7):
                nc.tensor.matmul(
                    wps[:, :NWIN], warm[:, :CHUNK], warm[:], start=True, stop=True
                )

            NTAIL = NCHUNK - 2 * NWIN  # 226
            for s in range(SEQ_PER_CORE):
                xb = s * XSTRIDE
                yb = s * NCHUNK
                # Two PSUM tags: 3x double-bank tiles for the 512-windows and
                # 2x single-bank tiles for the 226 tail = all 8 banks. Deep
                # rotation decouples TensorE from the evacuation copies.
                ps = pspool.tile([CHUNK, 2 * NWIN], f32, tag="big", bufs=3)
                pt = pspool.tile([CHUNK, NTAIL], f32, tag="small", bufs=2)
                for h0 in (0, CHUNK):  # H0 pass then H1 pass
                    xo = xb + (1 if h0 == 0 else 0)
                    st = h0 == 0
                    nc.tensor.matmul(
                        ps[:, 0:NWIN],
                        h_tile[:, h0 : h0 + CHUNK],
                        x_tile[:, xo : xo + NWIN],
                        start=st,
                        stop=not st,
                    )
                    nc.tensor.matmul(
                        ps[:, NWIN : 2 * NWIN],
                        h_tile[:, h0 : h0 + CHUNK],
                        x_tile[:, xo + NWIN : xo + 2 * NWIN],
                        start=st,
                        stop=not st,
                    )
                    nc.tensor.matmul(
                        pt[:, :],
                        h_tile[:, h0 : h0 + CHUNK],
                        x_tile[:, xo + 2 * NWIN : xo + NCHUNK],
                        start=st,
                        stop=not st,
                    )
                # Evacuate + cast f32->f16: scalar and vector on disjoint PSUM
                # banks; outputs ride the scalar HWDGE ring, whose
                # copy+dispatch pacing keeps the ring shallow so the final
                # transfer lands right after dispatch (no SWDGE drain tail).
                nc.scalar.copy(out=y_tile[:, yb : yb + NWIN], in_=ps[:, :NWIN])
                nc.vector.tensor_copy(
                    out=y_tile[:, yb + NWIN : yb + 2 * NWIN], in_=ps[:, NWIN:]
                )
                nc.vector.tensor_copy(
                    out=y_tile[:, yb + 2 * NWIN : yb + NCHUNK], in_=pt[:]
                )
                # Ship outputs two sequences per DMA (8 x 640 KB, all on the
                # scalar ring): the copies pace the stream and the shallow
                # HWDGE ring lands the final transfer right after dispatch.
                if s % 2 == 1:
                    nc.scalar.dma_start(
                        out=yt[:, yb - NCHUNK : yb + NCHUNK],
                        in_=y_tile[:, yb - NCHUNK : yb + NCHUNK],
                    )
    nc.compile()
    return nc


def _run_on_device(in_maps, trace=False):
    from concourse.bass_utils import run_bass_kernel_spmd

    if "nc" not in _NC_CACHE:
        _NC_CACHE["nc"] = _build_nc()
    return run_bass_kernel_spmd(
        _NC_CACHE["nc"], in_maps, core_ids=list(range(N_CORES)), trace=trace
    )


def _prepare_in_maps(x, b, a):
    hh = _build_h_matrices(b, a)
    xs = np.ascontiguousarray(np.asarray(x, np.float32).reshape(B_FULL, T_FULL))
    in_maps = []
    for c in range(N_CORES):
        xc = xs[c * SEQ_PER_CORE : (c + 1) * SEQ_PER_CORE]
        # phase-major: xt[m, s*1251 + 1 + c'] = x[s, c'*128 + m]; col s*1251 = 0
        xtc = np.zeros((CHUNK, SEQ_PER_CORE, XSTRIDE), np.float16)
        xtc[:, :, 1:] = xc.reshape(SEQ_PER_CORE, NCHUNK, CHUNK).transpose(2, 0, 1)
        in_maps.append({"xt": np.ascontiguousarray(xtc.reshape(CHUNK, XCOLS)), "hh": hh})
    return in_maps


def _assemble_output(results):
    out = np.empty((B_FULL, T_FULL, 1), np.float32)
    for c in range(N_CORES):
        ytc = np.asarray(results[c]["yt"])  # [128, 20000] f16 phase-major
        yc = (
            ytc.reshape(CHUNK, SEQ_PER_CORE, NCHUNK)
            .transpose(1, 2, 0)
            .reshape(SEQ_PER_CORE, T_FULL)
        )
        out[c * SEQ_PER_CORE : (c + 1) * SEQ_PER_CORE, :, 0] = yc.astype(np.float32)
    return out


def kernel(x, b, a):
    in_maps = _prepare_in_maps(x, b, a)
    res = _run_on_device(in_maps, trace=False)
    return _assemble_output(res.results)


def kernel_traced(x, b, a):
    """Same as kernel() but with neuron profiling; returns (output, exec_time_ns)."""
    in_maps = _prepare_in_maps(x, b, a)
    try:
        res = _run_on_device(in_maps, trace=True)
    except ModuleNotFoundError:
        res = _run_on_device(in_maps, trace=False)
    return _assemble_output(res.results), res.exec_time_ns


# revision 30
# speedup vs baseline: 1.0677x; 1.0677x over previous
"""Butterworth IIR (order 4) over [B=128, T=160000, 1] on 8 TRN2 NeuronCores.

Strategy: a stable IIR's impulse response decays geometrically (max pole
radius ~0.668 here), so the filter is numerically exactly (tail < 3e-23)
a 128-tap causal FIR:  y[t] = sum_{k<128} h[k] x[t-k].

Chunking time into 128-sample chunks, with X[c, m] = x[c*128 + m]:
    y[c*128 + j] = sum_m X[c, m] H0[m, j] + sum_m X[c-1, m] H1[m, j]
    H0[m, j] = h[j - m]        (0 <= j - m < 128)
    H1[m, j] = h[j - m + 128]  (0 <= j - m + 128 < 128)

On device this is two accumulating TensorE matmuls per window with the
small fixed H matrices as the stationary operand and a phase-major
(transposed) view of x as the wide moving operand (N up to 512 chunks):
    psum[j, n] = sum_m H0[m, j] XT[m, w+1+n] + sum_m H1[m, j] XT[m, w+n]
The host supplies XT (phase-major) with a leading zero column per
sequence so chunk 0 sees zeros as its predecessor, and un-transposes the
phase-major output.

This is HBM-bandwidth bound (~358 GB/s per core), so I/O is minimized:
f16 both ways (output cast f32->f16 on-chip during PSUM evacuation,
widened back to f32 on the host; rel-err budget 2e-2 vs ~4e-4 achieved)
and moved with few, large DMAs. PSUM evacuation alternates between the
Scalar and Vector engines so neither becomes the bottleneck; output DMAs
alternate between the scalar HWDGE ring and the gpsimd SWDGE ring while
the sync HWDGE ring streams the input.

Sharding: pure data-parallel, batch 128 -> 16 sequences per core.
"""

import numpy as np

B_FULL = 128
T_FULL = 160000
N_CORES = 8
SEQ_PER_CORE = B_FULL // N_CORES  # 16
CHUNK = 128
NCHUNK = T_FULL // CHUNK  # 1250
TAPS = 128
NWIN = 512  # matmul moving-operand width (chunks per matmul) = 1 PSUM bank
XSTRIDE = NCHUNK + 1  # per-seq column block in xt (leading zero column)
XCOLS = SEQ_PER_CORE * XSTRIDE  # 20016
YCOLS = SEQ_PER_CORE * NCHUNK  # 20000

_NC_CACHE = {}


def _impulse_response(b, a, n):
    """First n samples of the IIR impulse response, computed in float64
    via the same direct-form II transposed recurrence as the reference."""
    b = np.asarray(b, np.float64)
    a = np.asarray(a, np.float64)
    bn = b / a[0]
    an = a / a[0]
    order = len(a) - 1
    z = np.zeros(order, np.float64)
    h = np.zeros(n, np.float64)
    xt = 1.0
    for t in range(n):
        yt = bn[0] * xt + z[0]
        znew = np.empty_like(z)
        znew[:-1] = z[1:] + xt * bn[1:-1] - yt * an[1:-1]
        znew[-1] = xt * bn[-1] - yt * an[-1]
        z = znew
        h[t] = yt
        xt = 0.0
    return h


def _build_h_matrices(b, a):
    h = _impulse_response(b, a, TAPS)
    m = np.arange(CHUNK)[:, None]
    j = np.arange(CHUNK)[None, :]
    d0 = j - m
    d1 = j - m + CHUNK
    H0 = np.where((d0 >= 0) & (d0 < TAPS), h[np.clip(d0, 0, TAPS - 1)], 0.0)
    H1 = np.where((d1 >= 0) & (d1 < TAPS), h[np.clip(d1, 0, TAPS - 1)], 0.0)
    return np.concatenate([H0, H1], axis=1).astype(np.float16)  # [128, 256]


def _build_nc():
    import concourse.bacc as bacc
    import concourse.mybir as mybir
    from concourse.tile import TileContext

    f32 = mybir.dt.float32
    f16 = mybir.dt.float16
    nc = bacc.Bacc()
    xt = nc.declare_dram_parameter("xt", [CHUNK, XCOLS], f16, isOutput=False)
    hh = nc.declare_dram_parameter("hh", [CHUNK, 2 * CHUNK], f16, isOutput=False)
    yt = nc.declare_dram_parameter("yt", [CHUNK, YCOLS], f16, isOutput=True)

    with TileContext(nc) as tc:
        with (
            tc.tile_pool(name="const", bufs=1) as cpool,
            tc.tile_pool(name="xin", bufs=1) as xpool,
            tc.tile_pool(name="yout", bufs=1) as ypool,
            tc.tile_pool(name="warm", bufs=1) as wpool,
            tc.tile_pool(name="acc", bufs=2, space="PSUM") as pspool,
        ):
            h_tile = cpool.tile([CHUNK, 2 * CHUNK], f16)
            nc.sync.dma_start(out=h_tile[:], in_=hh[:])

            # Whole input and output stay resident in SBUF (~10.3 MB of 26).
            x_tile = xpool.tile([CHUNK, XCOLS], f16)
            y_tile = ypool.tile([CHUNK, YCOLS], f16)

            # Stream the input over the sync HWDGE ring (seqs 0-7) and the
            # gpsimd SWDGE ring (seqs 8-15). One HWDGE ring alone is limited
            # to ~250-300 GB/s (input takes ~20us and stalls late sequences),
            # and two HWDGE rings falsely couple through the shared DMAHW
            # completion lanes - but SWDGE completions tick a separate proc,
            # so this split halves input time without false ordering.
            nc.sync.dma_start(out=x_tile[:, : NWIN + 1], in_=xt[:, : NWIN + 1])
            nc.sync.dma_start(
                out=x_tile[:, NWIN + 1 : XSTRIDE], in_=xt[:, NWIN + 1 : XSTRIDE]
            )
            # Thick middle, thin tail: few big transfers keep the ring at
            # rate, while the final single-seq transfers land early so the
            # last sequences' compute isn't serialized behind a fat DMA.
            s0 = 1
            for nseq in (5, 6, 2, 1, 1):
                c0, c1 = s0 * XSTRIDE, (s0 + nseq) * XSTRIDE
                nc.sync.dma_start(out=x_tile[:, c0:c1], in_=xt[:, c0:c1])
                s0 += nseq

            # Warm the PE clock: the HAM up-clocks 1.2->2.4 GHz only after
            # ~3.4us of sustained activity, so burn dummy matmuls (no DMA
            # deps: operands come from a memset tile) while the first input
            # transfer is still in flight. Without this, the first two
            # sequences run at half clock and delay the whole pipeline.
            warm = wpool.tile([CHUNK, NWIN], f16)
            nc.vector.memset(warm[:], 0.0)
            wps = pspool.tile([CHUNK, 2 * NWIN], f32, tag="big", bufs=3)
            for _ in range(7):
                nc.tensor.matmul(
                    wps[:, :NWIN], warm[:, :CHUNK], warm[:], start=True, stop=True
                )

            NTAIL = NCHUNK - 2 * NWIN  # 226
            for s in range(SEQ_PER_CORE):
                xb = s * XSTRIDE
                yb = s * NCHUNK
                # Two PSUM tags: 3x double-bank tiles for the 512-windows and
                # 2x single-bank tiles for the 226 tail = all 8 banks. Deep
                # rotation decouples TensorE from the evacuation copies.
                ps = pspool.tile([CHUNK, 2 * NWIN], f32, tag="big", bufs=3)
                pt = pspool.tile([CHUNK, NTAIL], f32, tag="small", bufs=2)
                for h0 in (0, CHUNK):  # H0 pass then H1 pass
                    xo = xb + (1 if h0 == 0 else 0)
                    st = h0 == 0
                    nc.tensor.matmul(
                        ps[:, 0:NWIN],
                        h_tile[:, h0 : h0 + CHUNK],
                        x_tile[:, xo : xo + NWIN],
                        start=st,
                        stop=not st,
                    )
                    nc.tensor.matmul(
                        ps[:, NWIN : 2 * NWIN],
                        h_tile[:, h0 : h0 + CHUNK],
                        x_tile[:, xo + NWIN : xo + 2 * NWIN],
                        start=st,
                        stop=not st,
                    )
                    nc.tensor.matmul(
                        pt[:, :],
                        h_tile[:, h0 : h0 + CHUNK],
                        x_tile[:, xo + 2 * NWIN : xo + NCHUNK],
                        start=st,
                        stop=not st,
                    )
                # Evacuate + cast f32->f16: scalar and vector on disjoint PSUM
                # banks; outputs ride the scalar HWDGE ring, whose
                # copy+dispatch pacing keeps the ring shallow so the final
                # transfer lands right after dispatch (no SWDGE drain tail).
                nc.scalar.copy(out=y_tile[:, yb : yb + NWIN], in_=ps[:, :NWIN])
                nc.vector.tensor_copy(
                    out=y_tile[:, yb + NWIN : yb + 2 * NWIN], in_=ps[:, NWIN:]
                )
                nc.vector.tensor_copy(
                    out=y_tile[:, yb + 2 * NWIN : yb + NCHUNK], in_=pt[:]
                )
                # Ship outputs two sequences per DMA (8 x 640 KB): early ones
                # ride the gpsimd SWDGE ring (its 2-4us latency is harmless
                # mid-stream and it unloads the scalar ring), the last two ride
                # the shallow scalar HWDGE ring so the tail lands immediately.
                if s % 2 == 1:
                    out_eng = nc.gpsimd if s <= 11 else nc.scalar
                    out_eng.dma_start(
                        out=yt[:, yb - NCHUNK : yb + NCHUNK],
                        in_=y_tile[:, yb - NCHUNK : yb + NCHUNK],
                    )
    nc.compile()
    return nc


def _run_on_device(in_maps, trace=False):
    from concourse.bass_utils import run_bass_kernel_spmd

    if "nc" not in _NC_CACHE:
        _NC_CACHE["nc"] = _build_nc()
    return run_bass_kernel_spmd(
        _NC_CACHE["nc"], in_maps, core_ids=list(range(N_CORES)), trace=trace
    )


def _prepare_in_maps(x, b, a):
    hh = _build_h_matrices(b, a)
    xs = np.ascontiguousarray(np.asarray(x, np.float32).reshape(B_FULL, T_FULL))
    in_maps = []
    for c in range(N_CORES):
        xc = xs[c * SEQ_PER_CORE : (c + 1) * SEQ_PER_CORE]
        # phase-major: xt[m, s*1251 + 1 + c'] = x[s, c'*128 + m]; col s*1251 = 0
        xtc = np.zeros((CHUNK, SEQ_PER_CORE, XSTRIDE), np.float16)
        xtc[:, :, 1:] = xc.reshape(SEQ_PER_CORE, NCHUNK, CHUNK).transpose(2, 0, 1)
        in_maps.append({"xt": np.ascontiguousarray(xtc.reshape(CHUNK, XCOLS)), "hh": hh})
    return in_maps


def _assemble_output(results):
    out = np.empty((B_FULL, T_FULL, 1), np.float32)
    for c in range(N_CORES):
        ytc = np.asarray(results[c]["yt"])  # [128, 20000] f16 phase-major
        yc = (
            ytc.reshape(CHUNK, SEQ_PER_CORE, NCHUNK)
            .transpose(1, 2, 0)
            .reshape(SEQ_PER_CORE, T_FULL)
        )
        out[c * SEQ_PER_CORE : (c + 1) * SEQ_PER_CORE, :, 0] = yc.astype(np.float32)
    return out


def kernel(x, b, a):
    in_maps = _prepare_in_maps(x, b, a)
    res = _run_on_device(in_maps, trace=False)
    return _assemble_output(res.results)


def kernel_traced(x, b, a):
    """Same as kernel() but with neuron profiling; returns (output, exec_time_ns)."""
    in_maps = _prepare_in_maps(x, b, a)
    try:
        res = _run_on_device(in_maps, trace=True)
    except ModuleNotFoundError:
        res = _run_on_device(in_maps, trace=False)
    return _assemble_output(res.results), res.exec_time_ns


# revision 32
# speedup vs baseline: 1.1107x; 1.0403x over previous
"""Butterworth IIR (order 4) over [B=128, T=160000, 1] on 8 TRN2 NeuronCores.

Strategy: a stable IIR's impulse response decays geometrically (max pole
radius ~0.668 here), so the filter is numerically exactly (tail < 3e-23)
a 128-tap causal FIR:  y[t] = sum_{k<128} h[k] x[t-k].

Chunking time into 128-sample chunks, with X[c, m] = x[c*128 + m]:
    y[c*128 + j] = sum_m X[c, m] H0[m, j] + sum_m X[c-1, m] H1[m, j]
    H0[m, j] = h[j - m]        (0 <= j - m < 128)
    H1[m, j] = h[j - m + 128]  (0 <= j - m + 128 < 128)

On device this is two accumulating TensorE matmuls per window with the
small fixed H matrices as the stationary operand and a phase-major
(transposed) view of x as the wide moving operand (N up to 512 chunks):
    psum[j, n] = sum_m H0[m, j] XT[m, w+1+n] + sum_m H1[m, j] XT[m, w+n]
The host supplies XT (phase-major) with a leading zero column per
sequence so chunk 0 sees zeros as its predecessor, and un-transposes the
phase-major output.

This is HBM-bandwidth bound (~358 GB/s per core), so I/O is minimized:
f16 both ways (output cast f32->f16 on-chip during PSUM evacuation,
widened back to f32 on the host; rel-err budget 2e-2 vs ~4e-4 achieved)
and moved with few, large DMAs. PSUM evacuation alternates between the
Scalar and Vector engines so neither becomes the bottleneck; output DMAs
alternate between the scalar HWDGE ring and the gpsimd SWDGE ring while
the sync HWDGE ring streams the input.

Sharding: pure data-parallel, batch 128 -> 16 sequences per core.
"""

import numpy as np

B_FULL = 128
T_FULL = 160000
N_CORES = 8
SEQ_PER_CORE = B_FULL // N_CORES  # 16
CHUNK = 128
NCHUNK = T_FULL // CHUNK  # 1250
TAPS = 128
NWIN = 512  # matmul moving-operand width (chunks per matmul) = 1 PSUM bank
XSTRIDE = NCHUNK + 1  # per-seq column block in xt (leading zero column)
XCOLS = SEQ_PER_CORE * XSTRIDE  # 20016
YCOLS = SEQ_PER_CORE * NCHUNK  # 20000

_NC_CACHE = {}


def _impulse_response(b, a, n):
    """First n samples of the IIR impulse response, computed in float64
    via the same direct-form II transposed recurrence as the reference."""
    b = np.asarray(b, np.float64)
    a = np.asarray(a, np.float64)
    bn = b / a[0]
    an = a / a[0]
    order = len(a) - 1
    z = np.zeros(order, np.float64)
    h = np.zeros(n, np.float64)
    xt = 1.0
    for t in range(n):
        yt = bn[0] * xt + z[0]
        znew = np.empty_like(z)
        znew[:-1] = z[1:] + xt * bn[1:-1] - yt * an[1:-1]
        znew[-1] = xt * bn[-1] - yt * an[-1]
        z = znew
        h[t] = yt
        xt = 0.0
    return h


def _build_h_matrices(b, a):
    h = _impulse_response(b, a, TAPS)
    m = np.arange(CHUNK)[:, None]
    j = np.arange(CHUNK)[None, :]
    d0 = j - m
    d1 = j - m + CHUNK
    H0 = np.where((d0 >= 0) & (d0 < TAPS), h[np.clip(d0, 0, TAPS - 1)], 0.0)
    H1 = np.where((d1 >= 0) & (d1 < TAPS), h[np.clip(d1, 0, TAPS - 1)], 0.0)
    return np.concatenate([H0, H1], axis=1).astype(np.float16)  # [128, 256]


def _build_nc():
    import concourse.bacc as bacc
    import concourse.mybir as mybir
    from concourse.tile import TileContext

    f32 = mybir.dt.float32
    f16 = mybir.dt.float16
    nc = bacc.Bacc()
    xt = nc.declare_dram_parameter("xt", [CHUNK, XCOLS], f16, isOutput=False)
    hh = nc.declare_dram_parameter("hh", [CHUNK, 2 * CHUNK], f16, isOutput=False)
    yt = nc.declare_dram_parameter("yt", [CHUNK, YCOLS], f16, isOutput=True)

    with TileContext(nc) as tc:
        with (
            tc.tile_pool(name="const", bufs=1) as cpool,
            tc.tile_pool(name="xin", bufs=1) as xpool,
            tc.tile_pool(name="yout", bufs=1) as ypool,
            tc.tile_pool(name="warm", bufs=1) as wpool,
            tc.tile_pool(name="acc", bufs=2, space="PSUM") as pspool,
        ):
            h_tile = cpool.tile([CHUNK, 2 * CHUNK], f16)
            nc.sync.dma_start(out=h_tile[:], in_=hh[:])

            # Whole input and output stay resident in SBUF (~10.3 MB of 26).
            x_tile = xpool.tile([CHUNK, XCOLS], f16)
            y_tile = ypool.tile([CHUNK, YCOLS], f16)

            # Stream the input over the sync HWDGE ring (seqs 0-7) and the
            # gpsimd SWDGE ring (seqs 8-15). One HWDGE ring alone is limited
            # to ~250-300 GB/s (input takes ~20us and stalls late sequences),
            # and two HWDGE rings falsely couple through the shared DMAHW
            # completion lanes - but SWDGE completions tick a separate proc,
            # so this split halves input time without false ordering.
            nc.sync.dma_start(out=x_tile[:, : NWIN + 1], in_=xt[:, : NWIN + 1])
            nc.sync.dma_start(
                out=x_tile[:, NWIN + 1 : XSTRIDE], in_=xt[:, NWIN + 1 : XSTRIDE]
            )
            s0 = 1
            for nseq in (1, 2, 4, 4, 4):
                c0, c1 = s0 * XSTRIDE, (s0 + nseq) * XSTRIDE
                nc.sync.dma_start(out=x_tile[:, c0:c1], in_=xt[:, c0:c1])
                s0 += nseq

            # Warm the PE clock: the HAM up-clocks 1.2->2.4 GHz only after
            # ~3.4us of sustained activity, so burn dummy matmuls (no DMA
            # deps: operands come from a memset tile) while the first input
            # transfer is still in flight. Without this, the first two
            # sequences run at half clock and delay the whole pipeline.
            warm = wpool.tile([CHUNK, NWIN], f16)
            nc.vector.memset(warm[:], 0.0)
            wps = pspool.tile([CHUNK, 2 * NWIN], f32, tag="big", bufs=3)
            for _ in range(7):
                nc.tensor.matmul(
                    wps[:, :NWIN], warm[:, :CHUNK], warm[:], start=True, stop=True
                )

            NTAIL = NCHUNK - 2 * NWIN  # 226
            for s in range(SEQ_PER_CORE):
                xb = s * XSTRIDE
                yb = s * NCHUNK
                # Two PSUM tags: 3x double-bank tiles for the 512-windows and
                # 2x single-bank tiles for the 226 tail = all 8 banks. Deep
                # rotation decouples TensorE from the evacuation copies.
                ps = pspool.tile([CHUNK, 2 * NWIN], f32, tag="big", bufs=3)
                pt = pspool.tile([CHUNK, NTAIL], f32, tag="small", bufs=2)
                for h0 in (0, CHUNK):  # H0 pass then H1 pass
                    xo = xb + (1 if h0 == 0 else 0)
                    st = h0 == 0
                    nc.tensor.matmul(
                        ps[:, 0:NWIN],
                        h_tile[:, h0 : h0 + CHUNK],
                        x_tile[:, xo : xo + NWIN],
                        start=st,
                        stop=not st,
                    )
                    nc.tensor.matmul(
                        ps[:, NWIN : 2 * NWIN],
                        h_tile[:, h0 : h0 + CHUNK],
                        x_tile[:, xo + NWIN : xo + 2 * NWIN],
                        start=st,
                        stop=not st,
                    )
                    nc.tensor.matmul(
                        pt[:, :],
                        h_tile[:, h0 : h0 + CHUNK],
                        x_tile[:, xo + 2 * NWIN : xo + NCHUNK],
                        start=st,
                        stop=not st,
                    )
                # Evacuate + cast f32->f16: scalar and vector on disjoint PSUM
                # banks; outputs ride the scalar HWDGE ring, whose
                # copy+dispatch pacing keeps the ring shallow so the final
                # transfer lands right after dispatch (no SWDGE drain tail).
                nc.scalar.copy(out=y_tile[:, yb : yb + NWIN], in_=ps[:, :NWIN])
                nc.vector.tensor_copy(
                    out=y_tile[:, yb + NWIN : yb + 2 * NWIN], in_=ps[:, NWIN:]
                )
                nc.vector.tensor_copy(
                    out=y_tile[:, yb + 2 * NWIN : yb + NCHUNK], in_=pt[:]
                )
                # Ship outputs two sequences per DMA (8 x 640 KB, all on the
                # scalar ring): the copies pace the stream so the ring stays
                # shallow and the final transfer lands right after dispatch;
                # the gpsimd SWDGE path would add a multi-us drain tail.
                if s % 2 == 1:
                    nc.scalar.dma_start(
                        out=yt[:, yb - NCHUNK : yb + NCHUNK],
                        in_=y_tile[:, yb - NCHUNK : yb + NCHUNK],
                    )
    nc.compile()
    return nc


def _run_on_device(in_maps, trace=False):
    from concourse.bass_utils import run_bass_kernel_spmd

    if "nc" not in _NC_CACHE:
        _NC_CACHE["nc"] = _build_nc()
    return run_bass_kernel_spmd(
        _NC_CACHE["nc"], in_maps, core_ids=list(range(N_CORES)), trace=trace
    )


def _prepare_in_maps(x, b, a):
    hh = _build_h_matrices(b, a)
    xs = np.ascontiguousarray(np.asarray(x, np.float32).reshape(B_FULL, T_FULL))
    in_maps = []
    for c in range(N_CORES):
        xc = xs[c * SEQ_PER_CORE : (c + 1) * SEQ_PER_CORE]
        # phase-major: xt[m, s*1251 + 1 + c'] = x[s, c'*128 + m]; col s*1251 = 0
        xtc = np.zeros((CHUNK, SEQ_PER_CORE, XSTRIDE), np.float16)
        xtc[:, :, 1:] = xc.reshape(SEQ_PER_CORE, NCHUNK, CHUNK).transpose(2, 0, 1)
        in_maps.append({"xt": np.ascontiguousarray(xtc.reshape(CHUNK, XCOLS)), "hh": hh})
    return in_maps


def _assemble_output(results):
    out = np.empty((B_FULL, T_FULL, 1), np.float32)
    for c in range(N_CORES):
        ytc = np.asarray(results[c]["yt"])  # [128, 20000] f16 phase-major
        yc = (
            ytc.reshape(CHUNK, SEQ_PER_CORE, NCHUNK)
            .transpose(1, 2, 0)
            .reshape(SEQ_PER_CORE, T_FULL)
        )
        out[c * SEQ_PER_CORE : (c + 1) * SEQ_PER_CORE, :, 0] = yc.astype(np.float32)
    return out


def kernel(x, b, a):
    in_maps = _prepare_in_maps(x, b, a)
    res = _run_on_device(in_maps, trace=False)
    return _assemble_output(res.results)


def kernel_traced(x, b, a):
    """Same as kernel() but with neuron profiling; returns (output, exec_time_ns)."""
    in_maps = _prepare_in_maps(x, b, a)
    try:
        res = _run_on_device(in_maps, trace=True)
    except ModuleNotFoundError:
        res = _run_on_device(in_maps, trace=False)
    return _assemble_output(res.results), res.exec_time_ns
